# revision 9
# baseline (speedup 1.0000x reference)
"""Collision-cost (radius search) kernel for Trainium2, 8 NeuronCores.

Problem: for 960 query points (4x6x40 trajectory positions) against 50000
terrain points, count neighbors within radius 10 and sum their distances,
then per-query cost = -(mean_dist^2)/25 + 4 (0 if no neighbors), summed over
the 40 time steps -> (4, 6) output.

Strategy: spatial pruning + terrain sharding. The terrain is partitioned
into axis-aligned cells (greedy sweep: x-strips by terrain quantile, then
grow each cell in y until its margin-query count hits 128 or its terrain
count hits MAXT). A cell only needs the queries within distance 10 of its
box (margin test per axis), so each (cell, query-tile) "slot" is an
independent [<=128 queries x <=MAXT terrain] distance problem. Total
device work drops ~7x vs. all-pairs.

Slots are sorted by size and dealt round-robin to the 8 cores so every
core runs the same shape profile (SPMD). Per core the slots are packed
into [128, 2048] PSUM groups:

  TensorE : psum[q,m] = |q - t|^2 + eps   (K=7 augmented matmul per slot)
  ScalarE : d = sqrt(psum)                (ONE activation per 2048 group)
  VectorE : min(d,R)  accum -> su[slot];  (d<=R) accum -> cnt[slot]

Per-query neighbor sums/counts are then combined across cells on the host
(queries in several cells' margins get their partials added), and the tiny
per-query cost epilogue (960 values) also runs on the host.
"""

import os

import numpy as np

import concourse.bacc as bacc
import concourse.bass as bass
import concourse.mybir as mybir
import concourse.tile as tile
from concourse.bass_utils import run_bass_kernel_spmd

RQ = 5.0
THRESHOLD = 4.0
RADIUS = 2.0 * RQ  # 10.0

B, P, T = 4, 6, 40
Q = B * P * T  # 960
M = 50000
NCORES = 8
QPAD = 128
MARGIN = 10.1  # margin > RADIUS to cover fp16 coordinate rounding drift
MAXT = 896  # max terrain points per cell
NX = 8  # x-strips in the sweep partitioner
GROUP = 1536  # psum group width (3 banks), double buffered
NWARM = 6  # dummy matmuls at t~0 to finish the PE p-state ramp early
POOL_DMA0 = True  # leading DMA via Pool SWDGE (parallel with HWDGE path)
EPS = 0.02  # guards sqrt against fp32 cancellation making d^2 negative

f32 = mybir.dt.float32
f16 = mybir.dt.float16
bf16 = mybir.dt.bfloat16
# augmented contraction:
#   lhsT rows: [-2qx, -2qy, -2qz, 1, 1, q2h, q2l]
#   rhs  rows: [tx, ty, tz, t2h, t2l, 1, 1]
# so psum[q, m] = |q - t|^2 + eps exactly (for fp16-rounded coords), with the
# norm terms carried as exact fp16 hi/lo pairs.
KA = 7

FARQ = -140.0  # padding query coordinate (far from all terrain)
FART = 140.0  # padding terrain coordinate (far from all queries)

LAST_EXEC_TIME_NS = None
LAST_RESULTS = None

_CACHE = {}


def _partition(t, q):
    """Greedy sweep partition of terrain into cells with <=128 margin
    queries and <=MAXT terrain points. Returns list of slots
    (t_idx array, q_idx array(<=128))."""
    xs = np.quantile(t[:, 0], np.linspace(0, 1, NX + 1))
    xs[0], xs[-1] = -1e9, 1e9
    slots = []
    for i in range(NX):
        tmask = (t[:, 0] >= xs[i]) & (t[:, 0] < xs[i + 1])
        tidx_strip = np.where(tmask)[0]
        qxmask = (q[:, 0] >= xs[i] - MARGIN) & (q[:, 0] < xs[i + 1] + MARGIN)
        order = np.argsort(t[tidx_strip, 1], kind="stable")
        tidx_strip = tidx_strip[order]
        ty = t[tidx_strip, 1]
        n = len(tidx_strip)
        pos = 0
        y0 = -1e9
        while pos < n:
            lo_i, hi_i, best = pos + 1, n, pos + 1
            while lo_i <= hi_i:
                mid = (lo_i + hi_i) // 2
                yend = ty[mid - 1] + 1e-4 if mid < n else 1e9
                nq = (
                    qxmask
                    & (q[:, 1] >= y0 - MARGIN)
                    & (q[:, 1] < yend + MARGIN)
                ).sum()
                if (mid - pos) <= MAXT and nq <= QPAD:
                    best = mid
                    lo_i = mid + 1
                else:
                    hi_i = mid - 1
            yend = ty[best - 1] + 1e-4 if best < n else 1e9
            qsel = (
                qxmask & (q[:, 1] >= y0 - MARGIN) & (q[:, 1] < yend + MARGIN)
            )
            qidx = np.where(qsel)[0]
            cell_t = tidx_strip[pos:best]
            if len(qidx) <= QPAD:
                slots.append((cell_t, qidx))
            else:
                # dense pocket: duplicate the (small) terrain across several
                # query tiles
                nsplit = int(np.ceil(len(qidx) / QPAD))
                for part in np.array_split(qidx, nsplit):
                    slots.append((cell_t, part))
            pos = best
            y0 = yend
    return slots


def _pad128(n):
    return max(128, int(np.ceil(n / 128.0)) * 128)


def _plan(slots):
    """Harmonize slots into an SPMD plan: shared size profile, group
    packing (first-fit decreasing into <=GROUP psum bins, smallest bin
    first so the leading DMA+ACT are short), per-core slot assignment."""
    order = np.argsort([-len(s[0]) for s in slots], kind="stable")
    slots = [slots[i] for i in order]
    while len(slots) % NCORES:
        slots.append((np.empty(0, np.int64), np.empty(0, np.int64)))
    k = len(slots) // NCORES
    # rank group i = slots[8i:8i+8]; shared size = max padded cols in group
    sizes = [
        max(_pad128(len(slots[NCORES * i + c][0])) for c in range(NCORES))
        for i in range(k)
    ]
    # first-fit decreasing bin packing into psum groups of <= GROUP cols
    bins = []  # list of [ranks]
    bin_w = []
    for rank in range(k):  # sizes already descending
        s = sizes[rank]
        for b in range(len(bins)):
            if bin_w[b] + s <= GROUP:
                bins[b].append(rank)
                bin_w[b] += s
                break
        else:
            bins.append([rank])
            bin_w.append(s)
    # smallest bin first (short leading DMA/ACT), rest descending
    border = sorted(range(len(bins)), key=lambda b: bin_w[b])
    border = [border[0]] + sorted(border[1:], key=lambda b: -bin_w[b])
    groups = []  # list of (members=[(rank, size, offset_in_group)], width)
    for b in border:
        cur, cur_w = [], 0
        for rank in bins[b]:
            cur.append((rank, sizes[rank], cur_w))
            cur_w += sizes[rank]
        groups.append((cur, cur_w))
    percore = [[slots[NCORES * i + c] for i in range(k)] for c in range(NCORES)]
    return percore, sizes, groups


def _build_nc(sizes, groups):
    nslots = len(sizes)
    pcols = sum(gw for _, gw in groups)
    qcols = QPAD * nslots
    g0w = groups[0][1]
    nc = bacc.Bacc("TRN2", target_bir_lowering=False, debug=False)

    # single input tensor: [q_aug | group0 terrain | group1 terrain | ...]
    # so the leading DMA carries the queries plus the (small) first group
    data = nc.dram_tensor("data", [KA, qcols + pcols], f16, kind="ExternalInput")
    out = nc.dram_tensor("out", [QPAD, 2 * nslots], f32, kind="ExternalOutput")

    with tile.TileContext(nc) as tc:
        with (
            tc.tile_pool(name="singles", bufs=1) as singles,
            tc.tile_pool(name="trpool", bufs=3) as trpool,
            tc.tile_pool(name="pspool", bufs=2, space="PSUM") as pspool,
            tc.tile_pool(name="scratchps", bufs=1, space="PSUM") as scratchps,
            # one d slot per group: no reuse, so activations never carry a
            # WAR wait on the DVE readers (ACTIVATE allows only 1 sync wait)
            tc.tile_pool(name="dpool", bufs=len(groups)) as dpool,
            tc.tile_pool(name="wpool", bufs=2) as wpool,
            tc.tile_pool(name="spool", bufs=2) as spool,
            tc.tile_pool(name="smalls", bufs=1) as smalls,
        ):
            sb_first = singles.tile([KA, qcols + g0w], f16)
            dma0 = nc.gpsimd if POOL_DMA0 else nc.sync
            dma0.dma_start(out=sb_first, in_=data[:, : qcols + g0w])
            sb_qaug = sb_first[:, :qcols]

            out_parts = smalls.tile([QPAD, 2 * nslots], f32)

            # Warmup 1: load the Sqrt ACT table while DMAs stream in, so the
            # first real activation doesn't carry the table-load (and its
            # extra sync waits).
            warm = smalls.tile([QPAD, 1], f32)
            nc.vector.memset(warm, 1.0)
            nc.scalar.activation(
                out=warm,
                in_=warm,
                func=mybir.ActivationFunctionType.Sqrt,
            )
            # Warmup 2: dummy matmuls on a zeroed tile so the PE p-state ramp
            # (full clock only after ~3us of execution) completes while the
            # first DMA is still in flight.
            if NWARM:
                zt = smalls.tile([KA, 512], f16)
                nc.vector.memset(zt, 0.0)
                wps = scratchps.tile([QPAD, 512], f32)
                for _ in range(NWARM):
                    nc.tensor.matmul(
                        wps, zt[:, :QPAD], zt, start=True, stop=True
                    )

            goff = g0w
            for gi, (members, gw) in enumerate(groups):
                if gi == 0:
                    tr = sb_first[:, qcols : qcols + g0w]
                else:
                    trt = trpool.tile([KA, GROUP], f16, tag="tr")
                    nc.sync.dma_start(
                        out=trt[:, :gw],
                        in_=data[:, qcols + goff : qcols + goff + gw],
                    )
                    goff += gw
                    tr = trt[:, :gw]
                ps = pspool.tile([QPAD, GROUP], f32, tag="ps")
                for rank, s, off in members:
                    lhs = sb_qaug[:, rank * QPAD : (rank + 1) * QPAD]
                    # split [off, off+s) at psum bank (512) boundaries
                    a = off
                    while a < off + s:
                        b = min(off + s, (a // 512 + 1) * 512)
                        nc.tensor.matmul(
                            ps[:, a:b],
                            lhs,
                            tr[:, a:b],
                            start=True,
                            stop=True,
                        )
                        a = b
                d = dpool.tile([QPAD, GROUP], bf16, tag="d")
                nc.scalar.activation(
                    out=d[:, :gw],
                    in_=ps[:, :gw],
                    func=mybir.ActivationFunctionType.Sqrt,
                )
                w = wpool.tile([QPAD, GROUP], bf16, tag="w")
                s_ = spool.tile([QPAD, GROUP], bf16, tag="s")
                for rank, s, off in members:
                    # w = min(d, R); accum -> sum(min(d, R)) for this slot
                    nc.vector.tensor_scalar(
                        out=w[:, off : off + s],
                        in0=d[:, off : off + s],
                        scalar1=RADIUS,
                        scalar2=None,
                        op0=mybir.AluOpType.min,
                        op1=mybir.AluOpType.add,
                        accum_out=out_parts[:, rank : rank + 1],
                    )
                    # s = (d <= R); accum -> neighbor count for this slot
                    nc.vector.tensor_scalar(
                        out=s_[:, off : off + s],
                        in0=d[:, off : off + s],
                        scalar1=RADIUS,
                        scalar2=None,
                        op0=mybir.AluOpType.is_le,
                        op1=mybir.AluOpType.add,
                        accum_out=out_parts[:, nslots + rank : nslots + rank + 1],
                    )

            nc.sync.dma_start(out=out[:, :], in_=out_parts)

    nc.compile()
    return nc


def _aug_terrain(tpts):
    """[KA, n] fp16 augmented terrain columns from (n, 3) fp32 points."""
    n = tpts.shape[0]
    t16 = tpts.astype(np.float16)
    t32 = t16.astype(np.float32)
    t2 = (t32 * t32).sum(axis=1)
    t2h = t2.astype(np.float16)
    t2l = (t2 - t2h.astype(np.float32)).astype(np.float16)
    a = np.empty((KA, n), dtype=np.float16)
    a[:3] = t16.T
    a[3] = t2h
    a[4] = t2l
    a[5] = 1.0
    a[6] = 1.0
    return a


def _aug_queries(qpts):
    """[KA, n] fp16 augmented query rows from (n, 3) fp32 points."""
    n = qpts.shape[0]
    q16 = qpts.astype(np.float16)
    q32 = q16.astype(np.float32)
    a = np.empty((KA, n), dtype=np.float16)
    a[:3] = (-2.0 * q32.T).astype(np.float16)  # exact: 2*fp16 value
    a[3] = 1.0
    a[4] = 1.0
    q2 = (q32 * q32).sum(axis=1) + EPS
    q2h = q2.astype(np.float16)
    q2l = (q2 - q2h.astype(np.float32)).astype(np.float16)
    a[5] = q2h
    a[6] = q2l
    return a


def kernel(predicted_trajectories_global, terrain_points):
    global LAST_EXEC_TIME_NS, LAST_RESULTS
    traj = np.asarray(predicted_trajectories_global, dtype=np.float32)
    terrain = np.asarray(terrain_points, dtype=np.float32)
    assert traj.shape == (B, P, T, 3), traj.shape
    assert terrain.shape == (M, 3), terrain.shape

    q = np.ascontiguousarray(traj.reshape(-1, 3))
    slots = _partition(terrain, q)
    percore, sizes, groups = _plan(slots)
    nslots = len(sizes)
    pcols = sum(gw for _, gw in groups)

    key = (tuple(sizes), tuple(gw for _, gw in groups))
    if _CACHE.get("key") != key:
        _CACHE["nc"] = _build_nc(sizes, groups)
        _CACHE["key"] = key
    nc = _CACHE["nc"]

    # global slot column offsets within the terrain segment (same per core)
    slot_off = {}
    goff = 0
    for members, gw in groups:
        for rank, s, off in members:
            slot_off[rank] = goff + off
        goff += gw

    qcols = QPAD * nslots
    far_t = _aug_terrain(np.full((1, 3), FART, np.float32))
    far_q = _aug_queries(np.full((1, 3), FARQ, np.float32))

    in_maps = []
    slotmaps = []  # per core: (nslots, QPAD) int32 query ids, -1 = pad
    for c in range(NCORES):
        buf = np.empty((KA, qcols + pcols), np.float16)
        buf[:, :qcols] = far_q
        buf[:, qcols:] = far_t
        smap = np.full((nslots, QPAD), -1, np.int32)
        for rank in range(nslots):
            tidx, qidx = percore[c][rank]
            off = qcols + slot_off[rank]
            if len(tidx):
                buf[:, off : off + len(tidx)] = _aug_terrain(terrain[tidx])
            if len(qidx):
                buf[:, rank * QPAD : rank * QPAD + len(qidx)] = _aug_queries(
                    q[qidx]
                )
                smap[rank, : len(qidx)] = qidx
        in_maps.append({"data": np.ascontiguousarray(buf)})
        slotmaps.append(smap)

    trace = os.environ.get("KERNEL_TRACE", "0") == "1"
    res = run_bass_kernel_spmd(
        nc, in_maps, core_ids=list(range(NCORES)), trace=trace
    )
    LAST_EXEC_TIME_NS = res.exec_time_ns
    LAST_RESULTS = res

    dsum = np.zeros(Q, np.float64)
    cnt = np.zeros(Q, np.float64)
    for c in range(NCORES):
        o = res.results[c]["out"].reshape(QPAD, 2 * nslots).astype(np.float64)
        smap = slotmaps[c]
        for rank in range(nslots):
            valid = smap[rank] >= 0
            if not valid.any():
                continue
            qids = smap[rank][valid]
            su = o[valid, rank]
            cn = o[valid, nslots + rank]
            # su = dsum + R*(size - cnt)  =>  dsum = su + R*cnt - R*size
            dsum[qids] += su + RADIUS * cn - RADIUS * sizes[rank]
            cnt[qids] += cn

    d_mean = dsum / np.maximum(cnt, 1.0)
    per_point = np.where(cnt > 0, -(d_mean**2) / (RQ * RQ) + THRESHOLD, 0.0)
    return per_point.reshape(B, P, T).sum(axis=-1).astype(np.float32)


# revision 20
# speedup vs baseline: 1.0903x; 1.0903x over previous
"""Collision-cost (radius search) kernel for Trainium2, 8 NeuronCores.

Problem: for 960 query points (4x6x40 trajectory positions) against 50000
terrain points, count neighbors within radius 10 and sum their distances,
then per-query cost = -(mean_dist^2)/25 + 4 (0 if no neighbors), summed over
the 40 time steps -> (4, 6) output.

Strategy: spatial pruning + terrain sharding. The terrain is partitioned
into axis-aligned cells (greedy sweep: x-strips by terrain quantile, then
grow each cell in y until its margin-query count hits 128 or its terrain
count hits MAXT). A cell only needs the queries within distance 10 of its
box (margin test per axis), so each (cell, query-tile) "slot" is an
independent [<=128 queries x <=MAXT terrain] distance problem. Total
device work drops ~7x vs. all-pairs.

Slots are sorted by size and dealt round-robin to the 8 cores so every
core runs the same shape profile (SPMD). Per core the slots are packed
into [128, 2048] PSUM groups:

  TensorE : psum[q,m] = |q - t|^2 + eps   (K=7 augmented matmul per slot)
  ScalarE : d = sqrt(psum)                (ONE activation per 2048 group)
  VectorE : min(d,R)  accum -> su[slot];  (d<=R) accum -> cnt[slot]

Per-query neighbor sums/counts are then combined across cells on the host
(queries in several cells' margins get their partials added), and the tiny
per-query cost epilogue (960 values) also runs on the host.
"""

import os

import numpy as np

import concourse.bacc as bacc
import concourse.bass as bass
import concourse.mybir as mybir
import concourse.tile as tile
from concourse.bass_utils import run_bass_kernel_spmd

RQ = 5.0
THRESHOLD = 4.0
RADIUS = 2.0 * RQ  # 10.0

B, P, T = 4, 6, 40
Q = B * P * T  # 960
M = 50000
NCORES = 8
QPAD = 128
MARGIN = 10.1  # margin > RADIUS to cover fp16 coordinate rounding drift
MAXT = 1152  # max terrain points per cell
NX = 8  # x-strips in the sweep partitioner
GROUP = 1792  # psum group width (4 banks), double buffered
FIRSTBIN = 896  # target width of the leading (chain-starting) psum group
LASTBIN = False  # put the smallest slot in its own final group (short tail)
NWARM = 0  # dummy matmuls at t~0 to finish the PE p-state ramp early
POOL_DMA0 = False  # leading DMA via Pool SWDGE (parallel with HWDGE path)
EPS = 0.02  # guards sqrt against fp32 cancellation making d^2 negative

f32 = mybir.dt.float32
f16 = mybir.dt.float16
bf16 = mybir.dt.bfloat16
# augmented contraction:
#   lhsT rows: [-2qx, -2qy, -2qz, 1, 1, q2h, q2l]
#   rhs  rows: [tx, ty, tz, t2h, t2l, 1, 1]
# so psum[q, m] = |q - t|^2 + eps exactly (for fp16-rounded coords), with the
# norm terms carried as exact fp16 hi/lo pairs.
KA = 7

FARQ = -140.0  # padding query coordinate (far from all terrain)
FART = 140.0  # padding terrain coordinate (far from all queries)

LAST_EXEC_TIME_NS = None
LAST_RESULTS = None

_CACHE = {}


def _partition(t, q):
    """Greedy sweep partition of terrain into cells with <=128 margin
    queries and <=MAXT terrain points. Returns list of slots
    (t_idx array, q_idx array(<=128))."""
    xs = np.quantile(t[:, 0], np.linspace(0, 1, NX + 1))
    xs[0], xs[-1] = -1e9, 1e9
    slots = []
    for i in range(NX):
        tmask = (t[:, 0] >= xs[i]) & (t[:, 0] < xs[i + 1])
        tidx_strip = np.where(tmask)[0]
        qxmask = (q[:, 0] >= xs[i] - MARGIN) & (q[:, 0] < xs[i + 1] + MARGIN)
        order = np.argsort(t[tidx_strip, 1], kind="stable")
        tidx_strip = tidx_strip[order]
        ty = t[tidx_strip, 1]
        n = len(tidx_strip)
        pos = 0
        y0 = -1e9
        while pos < n:
            lo_i, hi_i, best = pos + 1, n, pos + 1
            while lo_i <= hi_i:
                mid = (lo_i + hi_i) // 2
                yend = ty[mid - 1] + 1e-4 if mid < n else 1e9
                nq = (
                    qxmask
                    & (q[:, 1] >= y0 - MARGIN)
                    & (q[:, 1] < yend + MARGIN)
                ).sum()
                if (mid - pos) <= MAXT and nq <= QPAD:
                    best = mid
                    lo_i = mid + 1
                else:
                    hi_i = mid - 1
            yend = ty[best - 1] + 1e-4 if best < n else 1e9
            qsel = (
                qxmask & (q[:, 1] >= y0 - MARGIN) & (q[:, 1] < yend + MARGIN)
            )
            qidx = np.where(qsel)[0]
            cell_t = tidx_strip[pos:best]
            if len(qidx) <= QPAD:
                slots.append((cell_t, qidx))
            else:
                # dense pocket: duplicate the (small) terrain across several
                # query tiles
                nsplit = int(np.ceil(len(qidx) / QPAD))
                for part in np.array_split(qidx, nsplit):
                    slots.append((cell_t, part))
            pos = best
            y0 = yend
    return slots


PAD = 32  # slot column granularity


def _pad128(n):
    return max(PAD, int(np.ceil(n / float(PAD))) * PAD)


def _plan(slots):
    """Harmonize slots into an SPMD plan: shared size profile, group
    packing (first-fit decreasing into <=GROUP psum bins, smallest bin
    first so the leading DMA+ACT are short), per-core slot assignment."""
    order = np.argsort([-len(s[0]) for s in slots], kind="stable")
    slots = [slots[i] for i in order]
    while len(slots) % NCORES:
        slots.append((np.empty(0, np.int64), np.empty(0, np.int64)))
    k = len(slots) // NCORES
    # rank group i = slots[8i:8i+8]; shared size = max padded cols in group
    sizes = [
        max(_pad128(len(slots[NCORES * i + c][0])) for c in range(NCORES))
        for i in range(k)
    ]
    # leading bin: greedily gather the smallest slots up to FIRSTBIN cols so
    # the chain-starting DMA + ACT are short; then first-fit decreasing into
    # psum groups of <= GROUP cols
    remaining = list(range(k))
    last_bin = []
    if LASTBIN and len(remaining) > 2:
        # smallest slot alone in the final group -> short post-ACT DVE tail
        lr = min(remaining, key=lambda r: sizes[r])
        last_bin = [lr]
        remaining = [r for r in remaining if r != lr]
    first_bin = []
    if FIRSTBIN:
        w = 0
        for rank in sorted(remaining, key=lambda r: sizes[r]):
            if w + sizes[rank] <= FIRSTBIN:
                first_bin.append(rank)
                w += sizes[rank]
        first_bin.sort(key=lambda r: -sizes[r])
        remaining = [r for r in remaining if r not in first_bin]
    bins = []  # list of [ranks]
    bin_w = []
    for rank in remaining:  # sizes already descending
        s = sizes[rank]
        for b in range(len(bins)):
            if bin_w[b] + s <= GROUP:
                bins[b].append(rank)
                bin_w[b] += s
                break
        else:
            bins.append([rank])
            bin_w.append(s)
    # first bin leads, middle descending by width, tiny bin last
    border = sorted(range(len(bins)), key=lambda b: -bin_w[b])
    bins = (
        ([first_bin] if first_bin else [])
        + [bins[b] for b in border]
        + ([last_bin] if last_bin else [])
    )
    border = range(len(bins))
    groups = []  # list of (members=[(rank, size, offset_in_group)], width)
    for b in border:
        cur, cur_w = [], 0
        for rank in bins[b]:
            cur.append((rank, sizes[rank], cur_w))
            cur_w += sizes[rank]
        groups.append((cur, cur_w))
    groups = [g for g in groups if g[1]]
    percore = [[slots[NCORES * i + c] for i in range(k)] for c in range(NCORES)]
    return percore, sizes, groups


def _build_nc(sizes, groups):
    nslots = len(sizes)
    pcols = sum(gw for _, gw in groups)
    qcols = QPAD * nslots
    g0w = groups[0][1]
    nc = bacc.Bacc("TRN2", target_bir_lowering=False, debug=False)

    # single input tensor: [q_aug | group0 terrain | group1 terrain | ...]
    # so the leading DMA carries the queries plus the (small) first group
    data = nc.dram_tensor("data", [KA, qcols + pcols], f16, kind="ExternalInput")
    out = nc.dram_tensor("out", [QPAD, 2 * nslots], f32, kind="ExternalOutput")

    with tile.TileContext(nc) as tc:
        with (
            tc.tile_pool(name="singles", bufs=1) as singles,
            tc.tile_pool(name="trpool", bufs=3) as trpool,
            tc.tile_pool(name="pspool", bufs=2, space="PSUM") as pspool,
            # one d slot per group: no reuse, so activations never carry a
            # WAR wait on the DVE readers (ACTIVATE allows only 1 sync wait)
            tc.tile_pool(name="dpool", bufs=len(groups)) as dpool,
            tc.tile_pool(name="wpool", bufs=2) as wpool,
            tc.tile_pool(name="spool", bufs=2) as spool,
            tc.tile_pool(name="smalls", bufs=1) as smalls,
        ):
            sb_first = singles.tile([KA, qcols + g0w], f16)
            dma0 = nc.gpsimd if POOL_DMA0 else nc.sync
            dma0.dma_start(out=sb_first, in_=data[:, : qcols + g0w])
            sb_qaug = sb_first[:, :qcols]

            out_parts = smalls.tile([QPAD, 2 * nslots], f32)

            # Warmup 1: load the Sqrt ACT table while DMAs stream in, so the
            # first real activation doesn't carry the table-load (and its
            # extra sync waits).
            warm = smalls.tile([QPAD, 1], f32)
            nc.vector.memset(warm, 1.0)
            nc.scalar.activation(
                out=warm,
                in_=warm,
                func=mybir.ActivationFunctionType.Sqrt,
            )
            # Warmup 2: dummy matmuls on a zeroed tile so the PE p-state ramp
            # (full clock only after ~3us of execution) completes while the
            # first DMA is still in flight.
            if NWARM:
                zt = smalls.tile([KA, 512], f16)
                nc.vector.memset(zt, 0.0)
                wps = pspool.tile([QPAD, 512], f32, tag="ps")
                for _ in range(NWARM):
                    nc.tensor.matmul(
                        wps, zt[:, :QPAD], zt, start=True, stop=True
                    )

            goff = g0w
            for gi, (members, gw) in enumerate(groups):
                if gi == 0:
                    tr = sb_first[:, qcols : qcols + g0w]
                else:
                    trt = trpool.tile([KA, GROUP], f16, tag="tr")
                    nc.sync.dma_start(
                        out=trt[:, :gw],
                        in_=data[:, qcols + goff : qcols + goff + gw],
                    )
                    goff += gw
                    tr = trt[:, :gw]
                ps = pspool.tile([QPAD, GROUP], f32, tag="ps")
                for rank, s, off in members:
                    lhs = sb_qaug[:, rank * QPAD : (rank + 1) * QPAD]
                    # split [off, off+s) at psum bank (512) boundaries
                    a = off
                    while a < off + s:
                        b = min(off + s, (a // 512 + 1) * 512)
                        nc.tensor.matmul(
                            ps[:, a:b],
                            lhs,
                            tr[:, a:b],
                            start=True,
                            stop=True,
                        )
                        a = b
                d = dpool.tile([QPAD, GROUP], bf16, tag="d")
                nc.scalar.activation(
                    out=d[:, :gw],
                    in_=ps[:, :gw],
                    func=mybir.ActivationFunctionType.Sqrt,
                )
                w = wpool.tile([QPAD, GROUP], bf16, tag="w")
                s_ = spool.tile([QPAD, GROUP], bf16, tag="s")
                for rank, s, off in members:
                    # w = min(d, R); accum -> sum(min(d, R)) for this slot
                    nc.vector.tensor_scalar(
                        out=w[:, off : off + s],
                        in0=d[:, off : off + s],
                        scalar1=RADIUS,
                        scalar2=None,
                        op0=mybir.AluOpType.min,
                        op1=mybir.AluOpType.add,
                        accum_out=out_parts[:, rank : rank + 1],
                    )
                    # s = (d <= R); accum -> neighbor count for this slot
                    nc.vector.tensor_scalar(
                        out=s_[:, off : off + s],
                        in0=d[:, off : off + s],
                        scalar1=RADIUS,
                        scalar2=None,
                        op0=mybir.AluOpType.is_le,
                        op1=mybir.AluOpType.add,
                        accum_out=out_parts[:, nslots + rank : nslots + rank + 1],
                    )

            nc.sync.dma_start(out=out[:, :], in_=out_parts)

    nc.compile()
    return nc


def _aug_terrain(tpts):
    """[KA, n] fp16 augmented terrain columns from (n, 3) fp32 points."""
    n = tpts.shape[0]
    t16 = tpts.astype(np.float16)
    t32 = t16.astype(np.float32)
    t2 = (t32 * t32).sum(axis=1)
    t2h = t2.astype(np.float16)
    t2l = (t2 - t2h.astype(np.float32)).astype(np.float16)
    a = np.empty((KA, n), dtype=np.float16)
    a[:3] = t16.T
    a[3] = t2h
    a[4] = t2l
    a[5] = 1.0
    a[6] = 1.0
    return a


def _aug_queries(qpts):
    """[KA, n] fp16 augmented query rows from (n, 3) fp32 points."""
    n = qpts.shape[0]
    q16 = qpts.astype(np.float16)
    q32 = q16.astype(np.float32)
    a = np.empty((KA, n), dtype=np.float16)
    a[:3] = (-2.0 * q32.T).astype(np.float16)  # exact: 2*fp16 value
    a[3] = 1.0
    a[4] = 1.0
    q2 = (q32 * q32).sum(axis=1) + EPS
    q2h = q2.astype(np.float16)
    q2l = (q2 - q2h.astype(np.float32)).astype(np.float16)
    a[5] = q2h
    a[6] = q2l
    return a


def kernel(predicted_trajectories_global, terrain_points):
    global LAST_EXEC_TIME_NS, LAST_RESULTS
    traj = np.asarray(predicted_trajectories_global, dtype=np.float32)
    terrain = np.asarray(terrain_points, dtype=np.float32)
    assert traj.shape == (B, P, T, 3), traj.shape
    assert terrain.shape == (M, 3), terrain.shape

    q = np.ascontiguousarray(traj.reshape(-1, 3))
    slots = _partition(terrain, q)
    percore, sizes, groups = _plan(slots)
    nslots = len(sizes)
    pcols = sum(gw for _, gw in groups)

    key = (tuple(sizes), tuple(gw for _, gw in groups))
    if _CACHE.get("key") != key:
        _CACHE["nc"] = _build_nc(sizes, groups)
        _CACHE["key"] = key
    nc = _CACHE["nc"]

    # global slot column offsets within the terrain segment (same per core)
    slot_off = {}
    goff = 0
    for members, gw in groups:
        for rank, s, off in members:
            slot_off[rank] = goff + off
        goff += gw

    qcols = QPAD * nslots
    far_t = _aug_terrain(np.full((1, 3), FART, np.float32))
    far_q = _aug_queries(np.full((1, 3), FARQ, np.float32))

    in_maps = []
    slotmaps = []  # per core: (nslots, QPAD) int32 query ids, -1 = pad
    for c in range(NCORES):
        buf = np.empty((KA, qcols + pcols), np.float16)
        buf[:, :qcols] = far_q
        buf[:, qcols:] = far_t
        smap = np.full((nslots, QPAD), -1, np.int32)
        for rank in range(nslots):
            tidx, qidx = percore[c][rank]
            off = qcols + slot_off[rank]
            if len(tidx):
                buf[:, off : off + len(tidx)] = _aug_terrain(terrain[tidx])
            if len(qidx):
                buf[:, rank * QPAD : rank * QPAD + len(qidx)] = _aug_queries(
                    q[qidx]
                )
                smap[rank, : len(qidx)] = qidx
        in_maps.append({"data": np.ascontiguousarray(buf)})
        slotmaps.append(smap)

    trace = os.environ.get("KERNEL_TRACE", "0") == "1"
    res = run_bass_kernel_spmd(
        nc, in_maps, core_ids=list(range(NCORES)), trace=trace
    )
    LAST_EXEC_TIME_NS = res.exec_time_ns
    LAST_RESULTS = res

    dsum = np.zeros(Q, np.float64)
    cnt = np.zeros(Q, np.float64)
    for c in range(NCORES):
        o = res.results[c]["out"].reshape(QPAD, 2 * nslots).astype(np.float64)
        smap = slotmaps[c]
        for rank in range(nslots):
            valid = smap[rank] >= 0
            if not valid.any():
                continue
            qids = smap[rank][valid]
            su = o[valid, rank]
            cn = o[valid, nslots + rank]
            # su = dsum + R*(size - cnt)  =>  dsum = su + R*cnt - R*size
            dsum[qids] += su + RADIUS * cn - RADIUS * sizes[rank]
            cnt[qids] += cn

    d_mean = dsum / np.maximum(cnt, 1.0)
    per_point = np.where(cnt > 0, -(d_mean**2) / (RQ * RQ) + THRESHOLD, 0.0)
    return per_point.reshape(B, P, T).sum(axis=-1).astype(np.float32)
